# revision 4
# baseline (speedup 1.0000x reference)
"""Trainium2 Bass kernel for nn_Attention_34351148434119 (8 NeuronCores).

Reference computation (faithful quirks included):
  q_proj = hid @ Wq; q, gate = split(q_proj)     # q is DEAD code downstream
  k = hid @ Wk; v = hid @ Wv                     # [B,KV,S,D]
  v = RoPE(v)  (k is NOT roped; q roped but unused)
  scores = (k @ v^T) * sqrt(D) + mask; attn = softmax_t(scores)   # per kv head
  out = (tile_G(attn @ v) * sigmoid(gate)) @ Wo

Sharding: core = b*4 + j  (b = batch, j = rank in 4-core batch group).
Per batch, S=2048 is split into 16 blocks of 128 rows; core j owns blocks
{j, 4+j, 8+j, 12+j} (slot k block = 4k+j) so every core has an identical
causal workload (uniform SPMD graph; per-core specialization only via
staged data).  v is shared within each batch group by two AllGathers
(d-major fp32 for scores, row-major bf16 for attn@v).

Precision: logits have sigma~105 (SCALING MULTIPLIES by sqrt(D)), so
softmax is effectively near-argmax and bf16 anywhere in the k/v->scores
chain flips argmax rows (rel err ~0.1).  The k/v projections use a
hi/lo bf16 split (3 passes = ~16-bit effective mantissa at full bf16 PE
speed); scores run in native fp32; gate / attn@v / out-proj run bf16.
Softmax is two-phase (parallel chunk maxima, then one exp wave with the
final max biased in via ACT's per-partition bias, 1/sum folded into a
per-partition scale on the [s,t]-layout attn before the PE transpose
to [t,s] for the attn@v contraction).
"""
import sys
import numpy as np

sys.path.insert(0, "/opt/trn_rl_repo")

B, S, HS = 2, 2048, 2048
H, KV, D = 16, 4, 128
G = H // KV
SCALING = float(D) ** 0.5
P = 128
NB = S // P            # 16 row blocks per batch
NCORES = 8
RANKS = 4              # cores per batch group
SLOTS = 4              # owned 128-row blocks per core
ROWS = SLOTS * P       # 512 rows per core
CHUNK = 512            # t-chunk = 4 t-tiles
NCHUNK = S // CHUNK    # 4
KT = HS // P           # 16 contraction tiles
NEG_THRESH = -1e8

_CACHE = {}


def _mask_classes(mask):
    """Classify each (s-slot k, t-chunk c) 512x512 region of the SxS mask.

    0 = skip (everything <= NEG_THRESH: contributes exact 0 after softmax)
    1 = plain (all zeros: no add needed)
    2 = add  (mixed: stage values and add on-chip)
    Slot k rows across all cores = blocks 4k..4k+3 = rows [512k, 512k+512).
    """
    cls = [[0] * NCHUNK for _ in range(SLOTS)]
    for k in range(SLOTS):
        for c in range(NCHUNK):
            reg = mask[512 * k:512 * (k + 1), 512 * c:512 * (c + 1)]
            if (reg <= NEG_THRESH).all():
                cls[k][c] = 0
            elif (reg == 0).all():
                cls[k][c] = 1
            else:
                cls[k][c] = 2
    ok = True
    for k in range(SLOTS):
        comp = [c for c in range(NCHUNK) if cls[k][c] != 0]
        # computed chunks must be a prefix starting at 0
        if comp != list(range(len(comp))) or 0 not in comp:
            ok = False
    if ok:
        # {k : chunk c computed} must be a suffix of slots for each c
        for c in range(NCHUNK):
            ks = [k for k in range(SLOTS) if cls[k][c] != 0]
            if ks != list(range(SLOTS - len(ks), SLOTS)):
                ok = False
    if not ok:
        # fully dense fallback: always correct for any mask
        cls = [[2] * NCHUNK for _ in range(SLOTS)]
    return cls


def _build(classes):
    from contextlib import ExitStack

    from concourse import bacc, mybir, tile
    from concourse.masks import make_identity

    f32 = mybir.dt.float32
    bf16 = mybir.dt.bfloat16
    Alu = mybir.AluOpType
    Act = mybir.ActivationFunctionType

    computed = [[c for c in range(NCHUNK) if classes[k][c] != 0] for k in range(SLOTS)]
    add_idx = {}
    for k in range(SLOTS):
        for c in range(NCHUNK):
            if classes[k][c] == 2:
                add_idx[(k, c)] = len(add_idx)
    n_add = max(len(add_idx), 1)
    dense = len(add_idx) > 6   # fallback / heavily-masked graph: favor SBUF

    nc = bacc.Bacc("TRN2", target_bir_lowering=False, debug=False,
                   num_devices=NCORES)

    f32r = mybir.dt.float32r
    hidT_d = nc.declare_dram_parameter("hidT", [HS, ROWS], f32r, isOutput=False)
    wqg_d = nc.declare_dram_parameter("wqg", [HS, HS], f32, isOutput=False)
    wk_d = nc.declare_dram_parameter("wk", [HS, KV * D], f32r, isOutput=False)
    wv_d = nc.declare_dram_parameter("wv", [HS, KV * D], f32r, isOutput=False)
    wo_d = nc.declare_dram_parameter("wo", [HS, HS], f32, isOutput=False)
    cosT_d = nc.declare_dram_parameter("cosT", [D, ROWS], f32, isOutput=False)
    sinT_d = nc.declare_dram_parameter("sinT", [D, ROWS], f32, isOutput=False)
    mask_d = nc.declare_dram_parameter("maskst", [n_add, P, CHUNK], f32,
                                       isOutput=False)
    out_d = nc.declare_dram_parameter("out", [ROWS, HS], f32, isOutput=True)

    rg = [[0, 1, 2, 3], [4, 5, 6, 7]]

    with tile.TileContext(nc) as tc, ExitStack() as ctx:
        sb = ctx.enter_context(tc.tile_pool(name="sb", bufs=2))
        ps = ctx.enter_context(tc.tile_pool(name="ps", bufs=8, space="PSUM"))
        dram = ctx.enter_context(tc.tile_pool(name="dram", bufs=1, space="DRAM"))

        # ---- constants ----
        id_f32 = sb.tile([P, P], f32, tag="c_idf")
        id_bf = sb.tile([P, P], bf16, tag="c_idb")
        make_identity(nc, id_f32[:])
        make_identity(nc, id_bf[:])
        cosT = sb.tile([D, ROWS], f32, tag="c_cos")
        sinT = sb.tile([D, ROWS], f32, tag="c_sin")
        nc.sync.dma_start(cosT[:], cosT_d[:, :])
        nc.sync.dma_start(sinT[:], sinT_d[:, :])

        # ---- k AND v projections in one pass over hidT (before the AGs) ----
        # fp32r matmul: hardware does the bf16 hi/lo split internally at
        # 1 cycle/row for free dim >= 256 (same speed as bf16, ~16-bit
        # effective mantissa) vs 4 cycles/row for plain fp32.
        pv = [ps.tile([P, ROWS], f32, tag="ps", name=f"pv{g}") for g in range(KV)]
        pk = [ps.tile([P, ROWS], f32, tag="ps", name=f"pk{g}") for g in range(KV)]
        hidb = []
        for kk in range(KT):
            hf = sb.tile([P, ROWS], f32r, tag="hidf", bufs=4)
            nc.sync.dma_start(hf[:], hidT_d[kk * P:(kk + 1) * P, :])
            wt = sb.tile([P, 2 * KV * D], f32r, tag="wkv", bufs=3)
            nc.sync.dma_start(wt[:, 0:KV * D], wv_d[kk * P:(kk + 1) * P, :])
            nc.sync.dma_start(wt[:, KV * D:], wk_d[kk * P:(kk + 1) * P, :])
            hb = sb.tile([P, ROWS], bf16, tag="bf16big", bufs=16)
            nc.scalar.copy(hb[:], hf[:].bitcast(f32))
            for g in range(KV):
                for pp, dst in ((0, pv[g]), (1, pk[g])):
                    sl = slice(pp * KV * D + g * P, pp * KV * D + (g + 1) * P)
                    nc.tensor.matmul(dst[:], wt[:, sl], hf[:],
                                     start=(kk == 0), stop=(kk == KT - 1))
            hidb.append(hb)

        kT = []   # per g: [128 d, 512 rows] f32, pre-scaled by sqrt(D)
        for g in range(KV):
            t = sb.tile([P, ROWS], f32, tag="kT", bufs=4)
            nc.scalar.mul(t[:], pk[g][:], SCALING)
            kT.append(t)

        vT = []   # per g: [128 d, 512 rows] f32, roped (in-place on vr)
        for g in range(KV):
            vr = sb.tile([P, ROWS], f32, tag="vraw", bufs=(2 if dense else 4))
            nc.scalar.copy(vr[:], pv[g][:])
            # RoPE: v' = v*cos + rot(v)*sin,  rot = [-v[64:], v[:64]]
            rot = sb.tile([P, ROWS], f32, tag="vrot", bufs=1)
            nc.vector.tensor_scalar_mul(rot[0:64, :], vr[64:128, :], -1.0)
            nc.vector.tensor_copy(rot[64:128, :], vr[0:64, :])
            nc.vector.tensor_mul(vr[:], vr[:], cosT[:])
            nc.vector.tensor_mul(rot[:], rot[:], sinT[:])
            nc.vector.tensor_add(vr[:], vr[:], rot[:])
            vT.append(vr)

        # ---- v row-major (bf16) via PE transpose ----
        vrow = []   # per rt: [128 rows, 512 d] bf16
        for rt in range(SLOTS):
            t = sb.tile([P, KV * D], bf16, tag="vrow", bufs=4)
            vrow.append(t)
        for g in range(KV):
            for rt in range(SLOTS):
                tp = ps.tile([P, P], f32, tag="ps")
                nc.tensor.transpose(tp[:], vT[g][:, rt * P:(rt + 1) * P], id_f32[:])
                nc.vector.tensor_copy(vrow[rt][:, g * P:(g + 1) * P], tp[:])

        # ---- AllGather v in both layouts (within 4-core batch group) ----
        vt_in = dram.tile([KV * D, ROWS], f32)
        vt_all_d = dram.tile([RANKS * KV * D, ROWS], f32)
        vr_in = dram.tile([ROWS, KV * D], bf16)
        vr_all_d = dram.tile([RANKS * ROWS, KV * D], bf16)
        for g in range(KV):
            nc.sync.dma_start(vt_in[g * P:(g + 1) * P, :], vT[g][:])
        for rt in range(SLOTS):
            nc.sync.dma_start(vr_in[rt * P:(rt + 1) * P, :], vrow[rt][:])
        nc.gpsimd.collective_compute(
            "AllGather", mybir.AluOpType.bypass, replica_groups=rg,
            ins=[vt_in.opt()], outs=[vt_all_d.opt()])
        nc.gpsimd.collective_compute(
            "AllGather", mybir.AluOpType.bypass, replica_groups=rg,
            ins=[vr_in.opt()], outs=[vr_all_d.opt()])

        # ---- load gathered v (gpsimd DMA queue, after gate weight DMAs) ----
        # vtc[g*NCHUNK+c]: [128 d(g), 512 t] f32, causal chunk c = blocks 4c..4c+3;
        # t-block 4c+r lives in rank r's AG chunk at column-slot c.
        vtc = []
        for g in range(KV):
            for c in range(NCHUNK):
                t = sb.tile([P, CHUNK], f32, tag="f32big", bufs=16,
                            name=f"vtc{g}_{c}")
                for r in range(RANKS):
                    nc.gpsimd.dma_start(
                        t[:, r * P:(r + 1) * P],
                        vt_all_d[r * KV * D + g * P:r * KV * D + (g + 1) * P,
                                 c * P:(c + 1) * P])
                vtc.append(t)
        vrg = []
        for i in range(RANKS * SLOTS):
            t = sb.tile([P, KV * D], bf16, tag="bf16big", bufs=16, name=f"vrg{i}")
            nc.gpsimd.dma_start(t[:], vr_all_d[i * P:(i + 1) * P, :])
            vrg.append(t)

        # ---- gate matmul (bf16) + fused sigmoid ----
        sigT = [None] * 16
        for nblk in range(4):
            wqb = []
            for kk in range(KT):
                fs = sb.tile([P, CHUNK], f32, tag="wslab", bufs=3, name=f"wqf{nblk}_{kk}")
                nc.sync.dma_start(
                    fs[:], wqg_d[kk * P:(kk + 1) * P, nblk * CHUNK:(nblk + 1) * CHUNK])
                bs = sb.tile([P, CHUNK], bf16, tag="wslabb", bufs=(8 if dense else 16), name=f"wqb{nblk}_{kk}")
                if kk % 2:
                    nc.vector.tensor_copy(bs[:], fs[:])
                else:
                    nc.scalar.copy(bs[:], fs[:])
                wqb.append(bs)
            for m in range(4):
                pg = ps.tile([P, ROWS], f32, tag="ps", name=f"pg{nblk}_{m}")
                for kk in range(KT):
                    nc.tensor.matmul(pg[:], wqb[kk][:, m * P:(m + 1) * P],
                                     hidb[kk][:], start=(kk == 0), stop=(kk == KT - 1))
                t = sb.tile([P, ROWS], bf16, tag="sigT", bufs=16, name=f"sig{nblk}_{m}")
                nc.scalar.activation(t[:], pg[:], Act.Sigmoid)
                sigT[nblk * 4 + m] = t


        # ---- prefetch first wo slab group (DMA-idle window in attention) ----
        wob0 = []
        for cc in range(KT):
            fs = sb.tile([P, CHUNK], f32, tag="wslab", bufs=3, name=f"wof{cc}")
            nc.sync.dma_start(fs[:], wo_d[cc * P:(cc + 1) * P, 0:CHUNK])
            bs = sb.tile([P, CHUNK], bf16, tag="wslabb", bufs=(8 if dense else 16), name=f"wob{cc}")
            if cc % 2:
                nc.vector.tensor_copy(bs[:], fs[:])
            else:
                nc.scalar.copy(bs[:], fs[:])
            wob0.append(bs)

        # ---- attention per kv head ----
        avT = [None] * KV
        for g in range(KV):
            # attnT tiles per t-block bi: [128 t, 512 s] bf16
            attnT = [sb.tile([P, ROWS], bf16, tag="attnT", bufs=16, name=f"attnT{g}_{bi}")
                     for bi in range(NB)]
            for k in range(SLOTS):
                comp = computed[k]
                nchk = len(comp)
                attn = sb.tile([P, CHUNK * nchk], bf16, tag="attn",
                               bufs=(1 if dense else 2),
                               padded_shape=[P, CHUNK * NCHUNK],
                               name=f"attn{g}_{k}")
                # two-phase softmax: all chunk matmuls + maxes run in
                # parallel, then one exp wave with the final max (no online
                # corrections needed).
                pscs = []
                cms = []
                for ci, c in enumerate(comp):
                    psc = ps.tile([P, CHUNK], f32, tag="ps", name=f"psc{ci}")
                    nc.tensor.matmul(psc[:], kT[g][:, k * P:(k + 1) * P],
                                     vtc[g * NCHUNK + c][:], start=True, stop=True)
                    if classes[k][c] == 2:
                        mt = sb.tile([P, CHUNK], f32, tag="msk", bufs=4,
                                     name=f"msk{g}_{k}_{c}")
                        nc.gpsimd.dma_start(mt[:], mask_d[add_idx[(k, c)], :, :])
                        nc.vector.tensor_add(psc[:], psc[:], mt[:])
                    cm = sb.tile([P, 1], f32, tag="stat", bufs=32, name=f"cm{ci}")
                    nc.vector.tensor_reduce(cm[:], psc[:], mybir.AxisListType.X,
                                            Alu.max, negate=True)
                    pscs.append(psc)
                    cms.append(cm)
                mneg = cms[0]   # -max
                for ci in range(1, nchk):
                    mnew = sb.tile([P, 1], f32, tag="stat", bufs=32, name=f"mn{ci}")
                    nc.vector.tensor_tensor(mnew[:], mneg[:], cms[ci][:], Alu.min)
                    mneg = mnew
                tot = None
                for ci in range(nchk):
                    csum = sb.tile([P, 1], f32, tag="stat", bufs=32, name=f"cs{ci}")
                    nc.scalar.activation(attn[:, ci * CHUNK:(ci + 1) * CHUNK],
                                         pscs[ci][:], Act.Exp, bias=mneg[:],
                                         accum_out=csum[:])
                    if tot is None:
                        tot = csum
                    else:
                        t2 = sb.tile([P, 1], f32, tag="stat", bufs=32, name=f"tt{ci}")
                        nc.vector.tensor_add(t2[:], tot[:], csum[:])
                        tot = t2
                rinv = sb.tile([P, 1], f32, tag="stat", bufs=32)
                nc.vector.reciprocal(rinv[:], tot[:])
                for ci in range(nchk):
                    nc.vector.tensor_scalar_mul(
                        attn[:, ci * CHUNK:(ci + 1) * CHUNK],
                        attn[:, ci * CHUNK:(ci + 1) * CHUNK], rinv[:])
                # transpose attn -> attnT column slot k
                for ci, c in enumerate(comp):
                    for i in range(4):
                        bi = 4 * c + i
                        tp = ps.tile([P, P], bf16, tag="ps")
                        nc.tensor.transpose(
                            tp[:], attn[:, ci * CHUNK + i * P:ci * CHUNK + (i + 1) * P],
                            id_bf[:])
                        if i % 2:
                            nc.scalar.copy(attnT[bi][:, k * P:(k + 1) * P], tp[:])
                        else:
                            nc.vector.tensor_copy(attnT[bi][:, k * P:(k + 1) * P], tp[:])
            # attn @ v  ->  avT[g] [128 d, 512 s]
            pav = ps.tile([P, ROWS], f32, tag="ps")
            first = True
            for bi in range(NB):
                ks = [k for k in range(SLOTS) if (bi // RANKS) in computed[k]]
                if not ks:
                    continue
                kmin = ks[0]
                lhs = vrg[(bi % RANKS) * SLOTS + (bi // RANKS)][:, g * P:(g + 1) * P]
                nc.tensor.matmul(pav[:, kmin * P:ROWS], lhs,
                                 attnT[bi][:, kmin * P:ROWS],
                                 start=first, stop=(bi == NB - 1))
                first = False
            t = sb.tile([P, ROWS], bf16, tag="avT", bufs=4)
            nc.vector.tensor_copy(t[:], pav[:])
            avT[g] = t


        # ---- gated = tile_G(avT) * sigT  (bf16) ----
        gat = []
        for g in range(KV):
            for i in range(G):
                t = sb.tile([P, ROWS], bf16, tag="gat", bufs=16)
                nc.vector.tensor_mul(t[:], avT[g][:], sigT[4 * g + i][:])
                gat.append(t)

        # ---- out projection (bf16) ----
        for nblk in range(4):
            if nblk == 0:
                wob = wob0
            else:
                wob = []
                for cc in range(KT):
                    fs = sb.tile([P, CHUNK], f32, tag="wslab", bufs=3)
                    nc.sync.dma_start(
                        fs[:], wo_d[cc * P:(cc + 1) * P, nblk * CHUNK:(nblk + 1) * CHUNK])
                    bs = sb.tile([P, CHUNK], bf16, tag="wslabb", bufs=(8 if dense else 16))
                    if cc % 2:
                        nc.vector.tensor_copy(bs[:], fs[:])
                    else:
                        nc.scalar.copy(bs[:], fs[:])
                    wob.append(bs)
            for rt in range(SLOTS):
                po = ps.tile([P, CHUNK], f32, tag="ps")
                for cc in range(KT):
                    nc.tensor.matmul(po[:], gat[cc][:, rt * P:(rt + 1) * P],
                                     wob[cc][:], start=(cc == 0), stop=(cc == KT - 1))
                t = sb.tile([P, CHUNK], f32, tag="oev", bufs=2)
                nc.scalar.copy(t[:], po[:])
                nc.sync.dma_start(
                    out_d[rt * P:(rt + 1) * P, nblk * CHUNK:(nblk + 1) * CHUNK], t[:])

    nc.compile()
    return nc


def kernel(hidden_states, cos, sin, attention_mask, Wq, Wk, Wv, Wo):
    from concourse.bass_utils import run_bass_kernel_spmd

    hidden_states = np.asarray(hidden_states, dtype=np.float32)
    cos = np.asarray(cos, dtype=np.float32)
    sin = np.asarray(sin, dtype=np.float32)
    mask = np.asarray(attention_mask, dtype=np.float32)[0, 0]
    Wq = np.asarray(Wq, dtype=np.float32)
    Wk = np.asarray(Wk, dtype=np.float32)
    Wv = np.asarray(Wv, dtype=np.float32)
    Wo = np.asarray(Wo, dtype=np.float32)

    classes = _mask_classes(mask)
    key = tuple(tuple(r) for r in classes)
    if key not in _CACHE:
        _CACHE[key] = _build(classes)
    nc = _CACHE[key]

    add_strips = []   # staged per core below; order must match build
    wqg = np.ascontiguousarray(Wq[:, HS:])

    in_maps = []
    for core in range(NCORES):
        b, j = divmod(core, RANKS)
        blocks = [RANKS * k + j for k in range(SLOTS)]
        rows = np.concatenate([np.arange(bi * P, (bi + 1) * P) for bi in blocks])
        strips = []
        for k in range(SLOTS):
            for c in range(NCHUNK):
                if classes[k][c] == 2:
                    bi = RANKS * k + j
                    strips.append(mask[bi * P:(bi + 1) * P,
                                       c * CHUNK:(c + 1) * CHUNK])
        if not strips:
            strips.append(np.zeros((P, CHUNK), np.float32))
        in_maps.append({
            "hidT": np.ascontiguousarray(hidden_states[b][rows].T),
            "wqg": wqg,
            "wk": Wk,
            "wv": Wv,
            "wo": Wo,
            "cosT": np.ascontiguousarray(cos[b][rows].T),
            "sinT": np.ascontiguousarray(sin[b][rows].T),
            "maskst": np.ascontiguousarray(np.stack(strips)),
        })

    res = run_bass_kernel_spmd(nc, in_maps, core_ids=list(range(NCORES)))

    out = np.empty((B, S, HS), np.float32)
    for core in range(NCORES):
        b, j = divmod(core, RANKS)
        o = res.results[core]["out"]
        for k in range(SLOTS):
            bi = RANKS * k + j
            out[b, bi * P:(bi + 1) * P, :] = o[k * P:(k + 1) * P, :]
    return out



# revision 8
# speedup vs baseline: 1.4109x; 1.4109x over previous
"""Trainium2 Bass kernel for nn_Attention_34351148434119 (8 NeuronCores).

Reference computation (faithful quirks included):
  q_proj = hid @ Wq; q, gate = split(q_proj)     # q is DEAD code downstream
  k = hid @ Wk; v = hid @ Wv                     # [B,KV,S,D]
  v = RoPE(v)  (k is NOT roped; q roped but unused)
  scores = (k @ v^T) * sqrt(D) + mask; attn = softmax_t(scores)   # per kv head
  out = (tile_G(attn @ v) * sigmoid(gate)) @ Wo

Sharding: core = b*4 + j  (b = batch, j = rank in 4-core batch group).
Per batch, S=2048 is split into 16 blocks of 128 rows; core j owns blocks
{j, 4+j, 8+j, 12+j} (slot k block = 4k+j) so every core has an identical
causal workload (uniform SPMD graph; per-core specialization only via
staged data).

Pipeline (v1 lessons: collectives starve host DMA queues while in
flight, and the PE clock ramps with uninterrupted streak length):
  1. v-projection only (fp32r: hw-internal bf16 hi/lo split at full PE
     speed), RoPE, stage, AllGather issued EARLY (~t=45us).
  2. k-projection (fp32r) from the resident hid tiles.
  3. Gate matmuls run bf16 from HOST-staged bf16 weights, fully
     preloaded into SBUF before the AG starts - zero DMA during the AG.
  4. Attention per kv head: fp32r scores (logits sigma~105: softmax is
     near-argmax, bf16 anywhere in k/v->scores flips rows), two-phase
     softmax, PE transposes, bf16 attn@v.  Row-major v is derived from
     the gathered d-major v by on-chip transposes (no 2nd AllGather).
  5. Gating + bf16 out-projection from host-staged bf16 Wo, streamed
     through a deep slab ring (no conversion ops).
"""
import sys
import numpy as np
import ml_dtypes

sys.path.insert(0, "/opt/trn_rl_repo")

B, S, HS = 2, 2048, 2048
H, KV, D = 16, 4, 128
G = H // KV
SCALING = float(D) ** 0.5
P = 128
NB = S // P            # 16 row blocks per batch
NCORES = 8
RANKS = 4              # cores per batch group
SLOTS = 4              # owned 128-row blocks per core
ROWS = SLOTS * P       # 512 rows per core
CHUNK = 512            # t-chunk = 4 t-tiles
NCHUNK = S // CHUNK    # 4
KT = HS // P           # 16 contraction tiles
KVD = KV * D
NEG_THRESH = -1e8

_CACHE = {}


def _mask_classes(mask):
    """Classify each (s-slot k, t-chunk c) 512x512 region of the SxS mask.

    0 = skip (everything <= NEG_THRESH: contributes exact 0 after softmax)
    1 = plain (all zeros: no add needed)
    2 = add  (mixed: stage values and add on-chip)
    Slot k rows across all cores = blocks 4k..4k+3 = rows [512k, 512k+512).
    """
    cls = [[0] * NCHUNK for _ in range(SLOTS)]
    for k in range(SLOTS):
        for c in range(NCHUNK):
            reg = mask[512 * k:512 * (k + 1), 512 * c:512 * (c + 1)]
            if (reg <= NEG_THRESH).all():
                cls[k][c] = 0
            elif (reg == 0).all():
                cls[k][c] = 1
            else:
                cls[k][c] = 2
    ok = True
    for k in range(SLOTS):
        comp = [c for c in range(NCHUNK) if cls[k][c] != 0]
        # computed chunks must be a prefix starting at 0
        if comp != list(range(len(comp))) or 0 not in comp:
            ok = False
    if ok:
        # {k : chunk c computed} must be a suffix of slots for each c
        for c in range(NCHUNK):
            ks = [k for k in range(SLOTS) if cls[k][c] != 0]
            if ks != list(range(SLOTS - len(ks), SLOTS)):
                ok = False
    if not ok:
        # fully dense fallback: always correct for any mask
        cls = [[2] * NCHUNK for _ in range(SLOTS)]
    return cls


def _mask_strips(mask, classes, j):
    """Per-core class-2 strips, in (k, c) scan order."""
    strips = []
    for k in range(SLOTS):
        for c in range(NCHUNK):
            if classes[k][c] == 2:
                bi = RANKS * k + j
                strips.append(np.ascontiguousarray(
                    mask[bi * P:(bi + 1) * P, c * CHUNK:(c + 1) * CHUNK]))
    return strips


def _dedup_map(mask, classes):
    """Map each class-2 (k,c) to a unique-strip index, valid for EVERY
    core (cores hold different rows, so strip equality must hold on all
    of them).  Returns (uniq_of_addidx, n_uniq) or None if coreswise
    inconsistent."""
    n_add = sum(1 for k in range(SLOTS) for c in range(NCHUNK)
                if classes[k][c] == 2)
    per_core = []
    for j in range(RANKS):
        strips = _mask_strips(mask, classes, j)
        uniq = []
        idx = []
        for s in strips:
            for ui, u in enumerate(uniq):
                if np.array_equal(s, u):
                    idx.append(ui)
                    break
            else:
                uniq.append(s)
                idx.append(len(uniq) - 1)
        per_core.append(tuple(idx))
    if len(set(per_core)) != 1:
        return tuple(range(n_add)), n_add     # no dedup
    return per_core[0], max(per_core[0]) + 1 if per_core[0] else 0


def _build(classes, uniq_idx, n_uniq):
    from contextlib import ExitStack

    from concourse import bacc, mybir, tile
    from concourse.masks import make_identity

    f32 = mybir.dt.float32
    f32r = mybir.dt.float32r
    bf16 = mybir.dt.bfloat16
    Alu = mybir.AluOpType
    Act = mybir.ActivationFunctionType

    computed = [[c for c in range(NCHUNK) if classes[k][c] != 0] for k in range(SLOTS)]
    add_idx = {}
    for k in range(SLOTS):
        for c in range(NCHUNK):
            if classes[k][c] == 2:
                add_idx[(k, c)] = len(add_idx)
    n_mask = max(n_uniq, 1)
    resident_mask = n_uniq <= 4

    nc = bacc.Bacc("TRN2", target_bir_lowering=False, debug=False,
                   num_devices=NCORES)

    hidT_d = nc.declare_dram_parameter("hidT", [HS, ROWS], f32r, isOutput=False)
    hidb_d = nc.declare_dram_parameter("hidb", [HS, ROWS], bf16, isOutput=False)
    wqg_d = nc.declare_dram_parameter("wqg", [HS, HS], bf16, isOutput=False)
    wk_d = nc.declare_dram_parameter("wk", [HS, KVD], f32r, isOutput=False)
    wv_d = nc.declare_dram_parameter("wv", [HS, KVD], f32r, isOutput=False)
    wo_d = nc.declare_dram_parameter("wo", [HS, HS], bf16, isOutput=False)
    cosT_d = nc.declare_dram_parameter("cosT", [D, ROWS], f32, isOutput=False)
    sinT_d = nc.declare_dram_parameter("sinT", [D, ROWS], f32, isOutput=False)
    mask_d = nc.declare_dram_parameter("maskst", [n_mask, P, CHUNK], f32,
                                       isOutput=False)
    out_d = nc.declare_dram_parameter("out", [ROWS, HS], f32, isOutput=True)

    rg = [[0, 1, 2, 3], [4, 5, 6, 7]]
    NSLAB = 64    # bf16 weight-slab ring (full gate preload, wo streams through)

    with tile.TileContext(nc) as tc, ExitStack() as ctx:
        sb = ctx.enter_context(tc.tile_pool(name="sb", bufs=2))
        ps = ctx.enter_context(tc.tile_pool(name="ps", bufs=6, space="PSUM"))
        ps2 = ctx.enter_context(tc.tile_pool(name="ps2", bufs=2, space="PSUM"))
        dram = ctx.enter_context(tc.tile_pool(name="dram", bufs=1, space="DRAM"))

        # ---- constants ----
        id_f32 = sb.tile([P, P], f32, tag="c_idf")
        id_bf = sb.tile([P, P], bf16, tag="c_idb")
        make_identity(nc, id_f32[:])
        make_identity(nc, id_bf[:])
        cosT = sb.tile([D, ROWS], f32, tag="c_cos")
        sinT = sb.tile([D, ROWS], f32, tag="c_sin")
        nc.sync.dma_start(cosT[:], cosT_d[:, :])
        nc.sync.dma_start(sinT[:], sinT_d[:, :])
        mtiles = []
        if resident_mask:
            for u in range(n_uniq):
                mt = sb.tile([P, CHUNK], f32, tag="msk", bufs=max(n_uniq, 1),
                             name=f"mt{u}")
                nc.sync.dma_start(mt[:], mask_d[u, :, :])
                mtiles.append(mt)

        # ---- v projection (fp32r), hid tiles kept resident for k ----
        pv = [ps.tile([P, ROWS], f32, tag="ps", name=f"pv{g}") for g in range(KV)]
        hid = []
        for kk in range(KT):
            hf = sb.tile([P, ROWS], f32r, tag="f32big", bufs=16, name=f"hf{kk}")
            nc.sync.dma_start(hf[:], hidT_d[kk * P:(kk + 1) * P, :])
            wt = sb.tile([P, KVD], f32r, tag="wkv", bufs=3, name=f"wv{kk}")
            nc.sync.dma_start(wt[:], wv_d[kk * P:(kk + 1) * P, :])
            for g in range(KV):
                nc.tensor.matmul(pv[g][:], wt[:, g * P:(g + 1) * P], hf[:],
                                 start=(kk == 0), stop=(kk == KT - 1))
            hid.append(hf)

        # ---- RoPE v (+ sqrt(D) score scaling folded into nothing: kT
        # keeps the scaling so v stays exact for attn@v) ----
        vT = []   # per g: [128 d, 512 rows] f32, roped
        for g in range(KV):
            vr = sb.tile([P, ROWS], f32, tag="vraw", bufs=4, name=f"vr{g}")
            nc.scalar.copy(vr[:], pv[g][:])
            rot = sb.tile([P, ROWS], f32, tag="vrot", bufs=2, name=f"rot{g}")
            nc.vector.tensor_scalar_mul(rot[0:64, :], vr[64:128, :], -1.0)
            nc.vector.tensor_copy(rot[64:128, :], vr[0:64, :])
            nc.vector.tensor_mul(vr[:], vr[:], cosT[:])
            nc.vector.tensor_mul(rot[:], rot[:], sinT[:])
            nc.vector.tensor_add(vr[:], vr[:], rot[:])
            vT.append(vr)

        # ---- stage + AllGather v (d-major fp32) EARLY ----
        vt_in = dram.tile([KVD, ROWS], f32)
        vt_all_d = dram.tile([RANKS * KVD, ROWS], f32)
        for g in range(KV):
            nc.gpsimd.dma_start(vt_in[g * P:(g + 1) * P, :], vT[g][:])
        nc.gpsimd.collective_compute(
            "AllGather", mybir.AluOpType.bypass, replica_groups=rg,
            ins=[vt_in.opt()], outs=[vt_all_d.opt()])

        # ---- k projection (fp32r) from resident hid tiles ----
        pk = [ps.tile([P, ROWS], f32, tag="ps", name=f"pk{g}") for g in range(KV)]
        for kk in range(KT):
            wt = sb.tile([P, KVD], f32r, tag="wkv", bufs=3, name=f"wk{kk}")
            nc.sync.dma_start(wt[:], wk_d[kk * P:(kk + 1) * P, :])
            for g in range(KV):
                nc.tensor.matmul(pk[g][:], wt[:, g * P:(g + 1) * P], hid[kk][:],
                                 start=(kk == 0), stop=(kk == KT - 1))

        kT = []   # per g: [128 d, 512 rows] f32r, pre-scaled by sqrt(D)
        for g in range(KV):
            t = sb.tile([P, ROWS], f32r, tag="kT", bufs=4, name=f"kT{g}")
            nc.scalar.mul(t[:], pk[g][:], SCALING)
            kT.append(t)

        # ---- hidb + ALL gate weight slabs preloaded (before AG hits) ----
        hidb = []
        for kk in range(KT):
            t = sb.tile([P, ROWS], bf16, tag="hidb", bufs=16, name=f"hb{kk}")
            nc.sync.dma_start(t[:], hidb_d[kk * P:(kk + 1) * P, :])
            hidb.append(t)
        wqb = []
        for nblk in range(4):
            for kk in range(KT):
                t = sb.tile([P, CHUNK], bf16, tag="wslab", bufs=NSLAB,
                            name=f"wq{nblk}_{kk}")
                nc.sync.dma_start(
                    t[:], wqg_d[kk * P:(kk + 1) * P, nblk * CHUNK:(nblk + 1) * CHUNK])
                wqb.append(t)

        # ---- gate matmul (bf16) + fused sigmoid: zero DMA during AG ----
        sigT = [None] * 16
        for nblk in range(4):
            for m in range(4):
                pg = ps.tile([P, ROWS], f32, tag="ps", name=f"pg{nblk}_{m}")
                for kk in range(KT):
                    nc.tensor.matmul(pg[:], wqb[nblk * KT + kk][:, m * P:(m + 1) * P],
                                     hidb[kk][:], start=(kk == 0), stop=(kk == KT - 1))
                t = sb.tile([P, ROWS], bf16, tag="sigT", bufs=16, name=f"sig{nblk}_{m}")
                nc.scalar.activation(t[:], pg[:], Act.Sigmoid)
                sigT[nblk * 4 + m] = t

        # ---- attention per kv head ----
        # vtc loaded per-g from the gathered buffer (gpsimd queue);
        # row-major bf16 v derived by PE transpose (no 2nd AllGather).
        avT = [None] * KV
        for g in range(KV):
            vtc = []   # per chunk c: [128 d, 512 t] f32r
            for c in range(NCHUNK):
                t = sb.tile([P, CHUNK], f32r, tag="f32big", bufs=16,
                            name=f"vtc{g}_{c}")
                for r in range(RANKS):
                    nc.gpsimd.dma_start(
                        t[:, r * P:(r + 1) * P],
                        vt_all_d[r * KVD + g * P:r * KVD + (g + 1) * P,
                                 c * P:(c + 1) * P].bitcast(f32r))
                vtc.append(t)
            vrow = []  # per t-block bi: [128 t, 128 d(g)] bf16
            for c in range(NCHUNK):
                for r in range(RANKS):
                    tp = ps2.tile([P, P], f32, tag="tp", name=f"tvp{g}_{c}_{r}")
                    nc.tensor.transpose(
                        tp[:], vtc[c][:, r * P:(r + 1) * P].bitcast(f32), id_f32[:])
                    t = sb.tile([P, P], bf16, tag="vrow", bufs=16,
                                name=f"vrow{g}_{c}_{r}")
                    if r % 2:
                        nc.scalar.copy(t[:], tp[:])
                    else:
                        nc.vector.tensor_copy(t[:], tp[:])
                    vrow.append(t)

            attnT = [sb.tile([P, ROWS], bf16, tag="attnT", bufs=16,
                             name=f"attnT{g}_{bi}")
                     for bi in range(NB)]
            for k in range(SLOTS):
                comp = computed[k]
                nchk = len(comp)
                attn = sb.tile([P, CHUNK * nchk], bf16, tag="attn", bufs=2,
                               padded_shape=[P, CHUNK * NCHUNK],
                               name=f"attn{g}_{k}")
                # two-phase softmax: all chunk matmuls + maxes run in
                # parallel, then one exp wave with the final max.
                pscs = []
                cms = []
                for ci, c in enumerate(comp):
                    psc = ps.tile([P, CHUNK], f32, tag="ps", name=f"psc{ci}")
                    nc.tensor.matmul(psc[:], kT[g][:, k * P:(k + 1) * P],
                                     vtc[c][:], start=True, stop=True)
                    if classes[k][c] == 2:
                        ai = add_idx[(k, c)]
                        if resident_mask:
                            mt = mtiles[uniq_idx[ai]]
                        else:
                            mt = sb.tile([P, CHUNK], f32, tag="msk", bufs=4,
                                         name=f"msk{g}_{k}_{c}")
                            nc.gpsimd.dma_start(mt[:], mask_d[ai, :, :])
                        nc.vector.tensor_add(psc[:], psc[:], mt[:])
                    cm = sb.tile([P, 1], f32, tag="stat", bufs=32, name=f"cm{ci}")
                    nc.vector.tensor_reduce(cm[:], psc[:], mybir.AxisListType.X,
                                            Alu.max, negate=True)
                    pscs.append(psc)
                    cms.append(cm)
                mneg = cms[0]   # -max
                for ci in range(1, nchk):
                    mnew = sb.tile([P, 1], f32, tag="stat", bufs=32, name=f"mn{ci}")
                    nc.vector.tensor_tensor(mnew[:], mneg[:], cms[ci][:], Alu.min)
                    mneg = mnew
                tot = None
                for ci in range(nchk):
                    csum = sb.tile([P, 1], f32, tag="stat", bufs=32, name=f"cs{ci}")
                    nc.scalar.activation(attn[:, ci * CHUNK:(ci + 1) * CHUNK],
                                         pscs[ci][:], Act.Exp, bias=mneg[:],
                                         accum_out=csum[:])
                    if tot is None:
                        tot = csum
                    else:
                        t2 = sb.tile([P, 1], f32, tag="stat", bufs=32, name=f"tt{ci}")
                        nc.vector.tensor_add(t2[:], tot[:], csum[:])
                        tot = t2
                rinv = sb.tile([P, 1], f32, tag="stat", bufs=32)
                nc.vector.reciprocal(rinv[:], tot[:])
                for ci in range(nchk):
                    nc.vector.tensor_scalar_mul(
                        attn[:, ci * CHUNK:(ci + 1) * CHUNK],
                        attn[:, ci * CHUNK:(ci + 1) * CHUNK], rinv[:])
                # transpose attn -> attnT column slot k
                for ci, c in enumerate(comp):
                    for i in range(4):
                        bi = 4 * c + i
                        tp = ps2.tile([P, P], bf16, tag="tp", name=f"tap{k}_{bi}")
                        nc.tensor.transpose(
                            tp[:], attn[:, ci * CHUNK + i * P:ci * CHUNK + (i + 1) * P],
                            id_bf[:])
                        if i % 2:
                            nc.scalar.copy(attnT[bi][:, k * P:(k + 1) * P], tp[:])
                        else:
                            nc.vector.tensor_copy(attnT[bi][:, k * P:(k + 1) * P], tp[:])
            # attn @ v  ->  avT[g] [128 d, 512 s]
            pav = ps.tile([P, ROWS], f32, tag="ps")
            first = True
            for bi in range(NB):
                ks = [k for k in range(SLOTS) if (bi // RANKS) in computed[k]]
                if not ks:
                    continue
                kmin = ks[0]
                nc.tensor.matmul(pav[:, kmin * P:ROWS], vrow[bi][:],
                                 attnT[bi][:, kmin * P:ROWS],
                                 start=first, stop=(bi == NB - 1))
                first = False
            t = sb.tile([P, ROWS], bf16, tag="avT", bufs=4)
            nc.vector.tensor_copy(t[:], pav[:])
            avT[g] = t

        # ---- gated = tile_G(avT) * sigT  (bf16, in place into sigT) ----
        gat = []
        for g in range(KV):
            for i in range(G):
                t = sigT[4 * g + i]
                nc.vector.tensor_mul(t[:], avT[g][:], t[:])
                gat.append(t)

        # ---- out projection (bf16, host-staged weights, deep ring) ----
        for nblk in range(4):
            wob = []
            for cc in range(KT):
                t = sb.tile([P, CHUNK], bf16, tag="wslab", bufs=NSLAB,
                            name=f"wo{nblk}_{cc}")
                nc.sync.dma_start(
                    t[:], wo_d[cc * P:(cc + 1) * P, nblk * CHUNK:(nblk + 1) * CHUNK])
                wob.append(t)
            for rt in range(SLOTS):
                po = ps.tile([P, CHUNK], f32, tag="ps")
                for cc in range(KT):
                    nc.tensor.matmul(po[:], gat[cc][:, rt * P:(rt + 1) * P],
                                     wob[cc][:], start=(cc == 0), stop=(cc == KT - 1))
                t = sb.tile([P, CHUNK], f32, tag="oev", bufs=2)
                nc.scalar.copy(t[:], po[:])
                nc.sync.dma_start(
                    out_d[rt * P:(rt + 1) * P, nblk * CHUNK:(nblk + 1) * CHUNK], t[:])

    nc.compile()
    return nc


def kernel(hidden_states, cos, sin, attention_mask, Wq, Wk, Wv, Wo):
    from concourse.bass_utils import run_bass_kernel_spmd

    hidden_states = np.asarray(hidden_states, dtype=np.float32)
    cos = np.asarray(cos, dtype=np.float32)
    sin = np.asarray(sin, dtype=np.float32)
    mask = np.asarray(attention_mask, dtype=np.float32)[0, 0]
    Wq = np.asarray(Wq, dtype=np.float32)
    Wk = np.asarray(Wk, dtype=np.float32)
    Wv = np.asarray(Wv, dtype=np.float32)
    Wo = np.asarray(Wo, dtype=np.float32)

    classes = _mask_classes(mask)
    uniq_idx, n_uniq = _dedup_map(mask, classes)
    key = (tuple(tuple(r) for r in classes), tuple(uniq_idx), n_uniq)
    if key not in _CACHE:
        _CACHE[key] = _build(classes, uniq_idx, n_uniq)
    nc = _CACHE[key]

    wqg = np.ascontiguousarray(Wq[:, HS:]).astype(ml_dtypes.bfloat16)
    wob = Wo.astype(ml_dtypes.bfloat16)

    in_maps = []
    for core in range(NCORES):
        b, j = divmod(core, RANKS)
        blocks = [RANKS * k + j for k in range(SLOTS)]
        rows = np.concatenate([np.arange(bi * P, (bi + 1) * P) for bi in blocks])
        strips = _mask_strips(mask, classes, j)
        if n_uniq > 0 and len(set(uniq_idx)) != len(strips):
            # staged per unique index
            uniq_strips = [None] * (max(uniq_idx) + 1)
            for si, ui in enumerate(uniq_idx):
                if uniq_strips[ui] is None:
                    uniq_strips[ui] = strips[si]
            strips = uniq_strips
        if not strips:
            strips = [np.zeros((P, CHUNK), np.float32)]
        hidT = np.ascontiguousarray(hidden_states[b][rows].T)
        in_maps.append({
            "hidT": hidT,
            "hidb": hidT.astype(ml_dtypes.bfloat16),
            "wqg": wqg,
            "wk": Wk,
            "wv": Wv,
            "wo": wob,
            "cosT": np.ascontiguousarray(cos[b][rows].T),
            "sinT": np.ascontiguousarray(sin[b][rows].T),
            "maskst": np.ascontiguousarray(np.stack(strips)),
        })

    res = run_bass_kernel_spmd(nc, in_maps, core_ids=list(range(NCORES)))

    out = np.empty((B, S, HS), np.float32)
    for core in range(NCORES):
        b, j = divmod(core, RANKS)
        o = res.results[core]["out"]
        for k in range(SLOTS):
            bi = RANKS * k + j
            out[b, bi * P:(bi + 1) * P, :] = o[k * P:(k + 1) * P, :]
    return out
